# revision 6
# baseline (speedup 1.0000x reference)
"""ATSS post-processor kernel for 8 TRN2 NeuronCores.

Data-parallel over the batch dim: core n handles image n. The device
kernel streams the dominant memory traffic (dot_product_logits,
15.6 MB/core) and computes the fused candidate scores
    fused[k, c] = sigmoid(dp[k, :]) @ pos_map[c, :]^T * sigmoid(ctr[k])
The cheap selection tail (top-1000, box decode, class-aware NMS,
top-100) runs on the host on the 1000 selected candidates per image.
"""

import numpy as np

import concourse.bass as bass
import concourse.bacc as bacc
import concourse.mybir as mybir
from concourse.tile import TileContext
from concourse.bass_utils import run_bass_kernel_spmd

N, C, L = 8, 80, 256
H, W = 100, 152
HW = H * W  # 15200
NT_FULL, P_LAST = 118, 96  # 15200 = 118*128 + 96
NTILES = 119
F32 = mybir.dt.float32

PRE_NMS_TOP_N = 1000
NMS_THRESH = 0.6
POST_TOP_N = 100
IMG_H, IMG_W = 800.0, 1216.0
BW_XY, BW_WH = 10.0, 5.0
MAX_DWH = np.float32(np.log(1000.0 / 16.0))
PRE_NMS_THRESH = 0.05

_CACHED = {}


def build_nc() -> bass.Bass:
    nc = bacc.Bacc()
    dp_e = nc.declare_dram_parameter("dp", [HW, L], F32, isOutput=False)
    ctr_e = nc.declare_dram_parameter("ctr", [HW], F32, isOutput=False)
    pm_e = nc.declare_dram_parameter("pm", [C, L], F32, isOutput=False)
    id_e = nc.declare_dram_parameter("ident", [128, 128], F32, isOutput=False)
    out_e = nc.declare_dram_parameter("fused", [HW, C], F32, isOutput=True)

    with TileContext(nc) as tc:
        with (
            tc.tile_pool(name="const", bufs=1) as cpool,
            tc.tile_pool(name="work", bufs=3) as wpool,
            tc.tile_pool(name="psum", bufs=2, space="PSUM") as ppool,
            tc.tile_pool(name="psum_sc", bufs=2, space="PSUM") as spool,
        ):
            ident = cpool.tile([128, 128], F32, tag="ident")
            nc.sync.dma_start(out=ident[:, :], in_=id_e[:, :])

            # centerness: rearrange "(f p) -> p f" so tile t's 128 locations
            # sit in column t; sigmoid once.
            ctr_raw = cpool.tile([128, NTILES], F32, tag="ctr_raw")
            ctr_sig = cpool.tile([128, NTILES], F32, tag="ctr_sig")
            nc.sync.dma_start(
                out=ctr_raw[:, 0:NT_FULL],
                in_=ctr_e[0 : NT_FULL * 128].rearrange("(a b) -> b a", b=128),
            )
            nc.sync.dma_start(
                out=ctr_raw[:P_LAST, NT_FULL : NT_FULL + 1],
                in_=ctr_e[NT_FULL * 128 : HW].rearrange("(a b) -> b a", b=P_LAST),
            )
            nc.scalar.activation(
                ctr_sig[:, 0:NT_FULL],
                ctr_raw[:, 0:NT_FULL],
                mybir.ActivationFunctionType.Sigmoid,
            )
            nc.scalar.activation(
                ctr_sig[:P_LAST, NT_FULL : NT_FULL + 1],
                ctr_raw[:P_LAST, NT_FULL : NT_FULL + 1],
                mybir.ActivationFunctionType.Sigmoid,
            )

            # pos_map -> pmT chunks [128 l, 80 c]
            pm_sb = cpool.tile([C, L], F32, tag="pm_sb")
            nc.sync.dma_start(out=pm_sb[:, :], in_=pm_e[:, :])
            pmT = []
            for h in range(2):
                ps = ppool.tile([128, 128], F32, tag="ps_tr")
                nc.tensor.matmul(
                    ps[:, :C],
                    pm_sb[:, 128 * h : 128 * (h + 1)],
                    ident[:C, :C],
                    is_transpose=True,
                )
                t = cpool.tile([128, C], F32, tag=f"pmT{h}")
                nc.vector.tensor_copy(t[:, :], ps[:, :C])
                pmT.append(t)

            # groups of up to 4 location-tiles (512 locations): transpose raw
            # dp on PE into PSUM, then ONE sigmoid per l-half fuses the
            # activation with the PSUM->SBUF eviction.
            GT = 4
            for g in range(0, NTILES, GT):
                tiles = [
                    (t, 128 if t < NT_FULL else P_LAST)
                    for t in range(g, min(g + GT, NTILES))
                ]
                gw = sum(p for _, p in tiles)  # group width in locations
                dps = []
                for j, (t, P) in enumerate(tiles):
                    dp_t = wpool.tile([128, L], F32, tag=f"dp_t{j}")
                    nc.sync.dma_start(
                        out=dp_t[:P, :], in_=dp_e[t * 128 : t * 128 + P, :]
                    )
                    dps.append(dp_t)
                sigT = []
                for h in range(2):
                    ps = ppool.tile([128, 512], F32, tag=f"ps_tr{h}")
                    off = 0
                    for j, (t, P) in enumerate(tiles):
                        nc.tensor.matmul(
                            ps[:, off : off + P],
                            dps[j][:P, 128 * h : 128 * (h + 1)],
                            ident[:P, :P],
                            is_transpose=True,
                        )
                        off += P
                    sg = wpool.tile([128, 512], F32, tag=f"sigT{h}")
                    nc.scalar.activation(
                        sg[:, :gw], ps[:, :gw],
                        mybir.ActivationFunctionType.Sigmoid,
                    )
                    sigT.append(sg)
                off = 0
                for j, (t, P) in enumerate(tiles):
                    ps_sc = spool.tile([128, C], F32, tag="ps_sc")
                    nc.tensor.matmul(
                        ps_sc[:P, :], sigT[0][:, off : off + P], pmT[0][:, :],
                        start=True, stop=False,
                    )
                    nc.tensor.matmul(
                        ps_sc[:P, :], sigT[1][:, off : off + P], pmT[1][:, :],
                        start=False, stop=True,
                    )
                    fused_t = wpool.tile([128, C], F32, tag="fused_t")
                    nc.vector.tensor_scalar_mul(
                        fused_t[:P, :], ps_sc[:P, :], ctr_sig[:P, t : t + 1]
                    )
                    nc.sync.dma_start(
                        out=out_e[t * 128 : t * 128 + P, :], in_=fused_t[:P, :]
                    )
                    off += P
    nc.compile()
    return nc


def run_device(dp_all: np.ndarray, ctr_all: np.ndarray, pm: np.ndarray,
               trace: bool = False):
    """dp_all [N,HW,L], ctr_all [N,HW] raw logits, pm [C,L] ->
    fused [N,HW,C] (float32). Returns (fused, BassKernelResults)."""
    if "nc" not in _CACHED:
        _CACHED["nc"] = build_nc()
    nc = _CACHED["nc"]
    ident = np.eye(128, dtype=np.float32)
    in_maps = [
        dict(
            dp=np.ascontiguousarray(dp_all[n], dtype=np.float32),
            ctr=np.ascontiguousarray(ctr_all[n], dtype=np.float32),
            pm=np.ascontiguousarray(pm, dtype=np.float32),
            ident=ident,
        )
        for n in range(N)
    ]
    res = run_bass_kernel_spmd(nc, in_maps, core_ids=list(range(N)), trace=trace)
    fused = np.stack([res.results[n]["fused"] for n in range(N)])
    return fused, res


def _host_tail(fused, ctr_sig, reg, anchors):
    """Per-image selection tail, float32 throughout to mirror the reference.
    fused [HW,C]; ctr_sig [HW]; reg [HW,4]; anchors [HW,4]."""
    flat = fused.reshape(-1)
    part = np.argpartition(-flat, PRE_NMS_TOP_N - 1)[:PRE_NMS_TOP_N]
    order = np.lexsort((part, -flat[part]))
    idx = part[order]
    vals = flat[idx]
    loc = idx // C
    labels = (idx % C + 1).astype(np.int32)
    # reference candidate mask: scores > 0.05 (scores = fused / ctr)
    scores_sel = vals / ctr_sig[loc]
    valid = scores_sel > PRE_NMS_THRESH

    a = anchors[loc]
    r = reg[loc]
    aw = a[:, 2] - a[:, 0]
    ah = a[:, 3] - a[:, 1]
    acx = a[:, 0] + np.float32(0.5) * aw
    acy = a[:, 1] + np.float32(0.5) * ah
    dx = r[:, 0] / np.float32(BW_XY)
    dy = r[:, 1] / np.float32(BW_XY)
    dw = np.minimum(r[:, 2] / np.float32(BW_WH), MAX_DWH)
    dh = np.minimum(r[:, 3] / np.float32(BW_WH), MAX_DWH)
    pcx = dx * aw + acx
    pcy = dy * ah + acy
    pw = np.exp(dw) * aw
    ph = np.exp(dh) * ah
    x1 = np.clip(pcx - np.float32(0.5) * pw, np.float32(0), np.float32(IMG_W))
    y1 = np.clip(pcy - np.float32(0.5) * ph, np.float32(0), np.float32(IMG_H))
    x2 = np.clip(pcx + np.float32(0.5) * pw, np.float32(0), np.float32(IMG_W))
    y2 = np.clip(pcy + np.float32(0.5) * ph, np.float32(0), np.float32(IMG_H))
    boxes = np.stack([x1, y1, x2, y2], -1)
    sc = np.sqrt(np.maximum(vals, np.float32(0)))
    valid = valid & (x2 - x1 > 0) & (y2 - y1 > 0)

    # class-aware greedy NMS on score-sorted candidates
    K = PRE_NMS_TOP_N
    s = np.where(valid, sc, np.float32(-1.0))
    ob = boxes + (labels.astype(np.float32) * np.float32(10000.0))[:, None]
    area = (ob[:, 2] - ob[:, 0]) * (ob[:, 3] - ob[:, 1])
    ix1 = np.maximum(ob[:, None, 0], ob[None, :, 0])
    iy1 = np.maximum(ob[:, None, 1], ob[None, :, 1])
    ix2 = np.minimum(ob[:, None, 2], ob[None, :, 2])
    iy2 = np.minimum(ob[:, None, 3], ob[None, :, 3])
    inter = np.maximum(ix2 - ix1, np.float32(0)) * np.maximum(
        iy2 - iy1, np.float32(0)
    )
    iou = inter / (area[:, None] + area[None, :] - inter + np.float32(1e-6))
    sup_mat = iou > np.float32(NMS_THRESH)
    keep = valid.copy()
    for i in range(K):
        if keep[i]:
            sup = sup_mat[i].copy()
            sup[: i + 1] = False
            keep &= ~sup
    s2 = np.where(keep, s, np.float32(-1.0))
    # top-100 of s2: s is descending so kept indices in order
    topi = np.lexsort((np.arange(K), -s2))[:POST_TOP_N]
    topv = s2[topi]
    good = topv > 0
    fb = np.where(good[:, None], boxes[topi], np.float32(0))
    fs = np.where(good, topv, np.float32(0)).astype(np.float32)
    fl = np.where(good, labels[topi], np.int32(0)).astype(np.int32)
    return fb.astype(np.float32), fs, fl


def kernel(box_regression, centerness, box_cls, dot_product_logits, anchors,
           pos_map):
    dp_all = np.asarray(dot_product_logits, dtype=np.float32)
    ctr_all = np.asarray(centerness, dtype=np.float32).reshape(N, HW)
    pm = np.asarray(pos_map, dtype=np.float32)
    br = np.asarray(box_regression, dtype=np.float32)
    anc = np.asarray(anchors, dtype=np.float32)

    fused, _ = run_device(dp_all, ctr_all, pm)

    ctr_sig = 1.0 / (1.0 + np.exp(-ctr_all, dtype=np.float32))
    out_b = np.zeros((N, POST_TOP_N, 4), np.float32)
    out_s = np.zeros((N, POST_TOP_N), np.float32)
    out_l = np.zeros((N, POST_TOP_N), np.int32)
    for n in range(N):
        reg = br[n].transpose(1, 2, 0).reshape(HW, 4)
        fb, fs, fl = _host_tail(fused[n], ctr_sig[n], reg, anc)
        out_b[n], out_s[n], out_l[n] = fb, fs, fl
    return out_b, out_s, out_l


# revision 10
# speedup vs baseline: 1.3033x; 1.3033x over previous
"""ATSS post-processor kernel for 8 TRN2 NeuronCores.

Data-parallel over the batch dim: core n handles image n. The device
kernel streams the dominant memory traffic (dot_product_logits,
15.6 MB/core) and computes the fused candidate scores
    fused[k, c] = sigmoid(dp[k, :]) @ pos_map[c, :]^T * sigmoid(ctr[k])
The cheap selection tail (top-1000, box decode, class-aware NMS,
top-100) runs on the host on the 1000 selected candidates per image.
"""

import numpy as np

import concourse.bass as bass
import concourse.bacc as bacc
import concourse.mybir as mybir
from concourse.tile import TileContext
from concourse.bass_utils import run_bass_kernel_spmd

N, C, L = 8, 80, 256
H, W = 100, 152
HW = H * W  # 15200
NT_FULL, P_LAST = 118, 96  # 15200 = 118*128 + 96
NTILES = 119
F32 = mybir.dt.float32

PRE_NMS_TOP_N = 1000
NMS_THRESH = 0.6
POST_TOP_N = 100
IMG_H, IMG_W = 800.0, 1216.0
BW_XY, BW_WH = 10.0, 5.0
MAX_DWH = np.float32(np.log(1000.0 / 16.0))
PRE_NMS_THRESH = 0.05

_CACHED = {}


def build_nc() -> bass.Bass:
    nc = bacc.Bacc()
    dp_e = nc.declare_dram_parameter("dp", [HW, L], F32, isOutput=False)
    ctr_e = nc.declare_dram_parameter("ctr", [HW], F32, isOutput=False)
    pm_e = nc.declare_dram_parameter("pm", [C, L], F32, isOutput=False)
    id_e = nc.declare_dram_parameter("ident", [128, 128], F32, isOutput=False)
    out_e = nc.declare_dram_parameter("fused", [HW, C], F32, isOutput=True)

    with TileContext(nc) as tc:
        with (
            tc.tile_pool(name="const", bufs=1) as cpool,
            tc.tile_pool(name="work", bufs=4) as wpool,
            tc.tile_pool(name="psum", bufs=2, space="PSUM") as ppool,
            tc.tile_pool(name="psum_sc", bufs=2, space="PSUM") as spool,
        ):
            ident = cpool.tile([128, 128], F32, tag="ident")
            nc.sync.dma_start(out=ident[:, :], in_=id_e[:, :])

            # centerness: rearrange "(f p) -> p f" so tile t's 128 locations
            # sit in column t; sigmoid once.
            ctr_raw = cpool.tile([128, NTILES], F32, tag="ctr_raw")
            ctr_sig = cpool.tile([128, NTILES], F32, tag="ctr_sig")
            nc.sync.dma_start(
                out=ctr_raw[:, 0:NT_FULL],
                in_=ctr_e[0 : NT_FULL * 128].rearrange("(a b) -> b a", b=128),
            )
            nc.sync.dma_start(
                out=ctr_raw[:P_LAST, NT_FULL : NT_FULL + 1],
                in_=ctr_e[NT_FULL * 128 : HW].rearrange("(a b) -> b a", b=P_LAST),
            )
            nc.scalar.activation(
                ctr_sig[:, 0:NT_FULL],
                ctr_raw[:, 0:NT_FULL],
                mybir.ActivationFunctionType.Sigmoid,
            )
            nc.scalar.activation(
                ctr_sig[:P_LAST, NT_FULL : NT_FULL + 1],
                ctr_raw[:P_LAST, NT_FULL : NT_FULL + 1],
                mybir.ActivationFunctionType.Sigmoid,
            )

            # pos_map -> pmT chunks [128 l, 80 c]
            pm_sb = cpool.tile([C, L], F32, tag="pm_sb")
            nc.sync.dma_start(out=pm_sb[:, :], in_=pm_e[:, :])
            pmT = []
            for h in range(2):
                ps = ppool.tile([128, 128], F32, tag="ps_tr")
                nc.tensor.matmul(
                    ps[:, :C],
                    pm_sb[:, 128 * h : 128 * (h + 1)],
                    ident[:C, :C],
                    is_transpose=True,
                )
                t = cpool.tile([128, C], F32, tag=f"pmT{h}")
                nc.vector.tensor_copy(t[:, :], ps[:, :C])
                pmT.append(t)

            # groups of up to 4 location-tiles (512 locations): transpose raw
            # dp on PE into PSUM, then ONE sigmoid per l-half fuses the
            # activation with the PSUM->SBUF eviction.
            GT = 4
            for g in range(0, NTILES, GT):
                tiles = [
                    (t, 128 if t < NT_FULL else P_LAST)
                    for t in range(g, min(g + GT, NTILES))
                ]
                gw = sum(p for _, p in tiles)  # group width in locations
                dps = []
                for j, (t, P) in enumerate(tiles):
                    dp_t = wpool.tile([128, L], F32, tag=f"dp_t{j}")
                    nc.sync.dma_start(
                        out=dp_t[:P, :], in_=dp_e[t * 128 : t * 128 + P, :]
                    )
                    dps.append(dp_t)
                sigT = []
                for h in range(2):
                    ps = ppool.tile([128, 512], F32, tag=f"ps_tr{h}")
                    off = 0
                    for j, (t, P) in enumerate(tiles):
                        nc.tensor.matmul(
                            ps[:, off : off + P],
                            dps[j][:P, 128 * h : 128 * (h + 1)],
                            ident[:P, :P],
                            is_transpose=True,
                        )
                        off += P
                    sg = wpool.tile([128, 512], F32, tag=f"sigT{h}")
                    nc.scalar.activation(
                        sg[:, :gw], ps[:, :gw],
                        mybir.ActivationFunctionType.Sigmoid,
                    )
                    sigT.append(sg)
                fused_g = wpool.tile([128, GT * C], F32, tag="fused_g")
                off = 0
                for j, (t, P) in enumerate(tiles):
                    ps_sc = spool.tile([128, C], F32, tag="ps_sc")
                    nc.tensor.matmul(
                        ps_sc[:P, :], sigT[0][:, off : off + P], pmT[0][:, :],
                        start=True, stop=False,
                    )
                    nc.tensor.matmul(
                        ps_sc[:P, :], sigT[1][:, off : off + P], pmT[1][:, :],
                        start=False, stop=True,
                    )
                    nc.vector.tensor_scalar_mul(
                        fused_g[:P, j * C : (j + 1) * C],
                        ps_sc[:P, :],
                        ctr_sig[:P, t : t + 1],
                    )
                    off += P
                # one grouped store: DRAM rows [g*128, g*128+gw) <-
                # SBUF [p, (j, c)] laid out as "(j p) c -> p (j c)"
                if len(tiles) == GT and gw == GT * 128:
                    nc.sync.dma_start(
                        out=out_e[g * 128 : g * 128 + gw, :].rearrange(
                            "(j p) c -> p j c", p=128
                        ),
                        in_=fused_g[:, :].rearrange("p (j c) -> p j c", c=C),
                    )
                else:
                    for j, (t, P) in enumerate(tiles):
                        nc.sync.dma_start(
                            out=out_e[t * 128 : t * 128 + P, :],
                            in_=fused_g[:P, j * C : (j + 1) * C],
                        )
    nc.compile()
    return nc


def run_device(dp_all: np.ndarray, ctr_all: np.ndarray, pm: np.ndarray,
               trace: bool = False):
    """dp_all [N,HW,L], ctr_all [N,HW] raw logits, pm [C,L] ->
    fused [N,HW,C] (float32). Returns (fused, BassKernelResults)."""
    if "nc" not in _CACHED:
        _CACHED["nc"] = build_nc()
    nc = _CACHED["nc"]
    ident = np.eye(128, dtype=np.float32)
    in_maps = [
        dict(
            dp=np.ascontiguousarray(dp_all[n], dtype=np.float32),
            ctr=np.ascontiguousarray(ctr_all[n], dtype=np.float32),
            pm=np.ascontiguousarray(pm, dtype=np.float32),
            ident=ident,
        )
        for n in range(N)
    ]
    res = run_bass_kernel_spmd(nc, in_maps, core_ids=list(range(N)), trace=trace)
    fused = np.stack([res.results[n]["fused"] for n in range(N)])
    return fused, res


def _host_tail(fused, ctr_sig, reg, anchors):
    """Per-image selection tail, float32 throughout to mirror the reference.
    fused [HW,C]; ctr_sig [HW]; reg [HW,4]; anchors [HW,4]."""
    flat = fused.reshape(-1)
    part = np.argpartition(-flat, PRE_NMS_TOP_N - 1)[:PRE_NMS_TOP_N]
    order = np.lexsort((part, -flat[part]))
    idx = part[order]
    vals = flat[idx]
    loc = idx // C
    labels = (idx % C + 1).astype(np.int32)
    # reference candidate mask: scores > 0.05 (scores = fused / ctr)
    scores_sel = vals / ctr_sig[loc]
    valid = scores_sel > PRE_NMS_THRESH

    a = anchors[loc]
    r = reg[loc]
    aw = a[:, 2] - a[:, 0]
    ah = a[:, 3] - a[:, 1]
    acx = a[:, 0] + np.float32(0.5) * aw
    acy = a[:, 1] + np.float32(0.5) * ah
    dx = r[:, 0] / np.float32(BW_XY)
    dy = r[:, 1] / np.float32(BW_XY)
    dw = np.minimum(r[:, 2] / np.float32(BW_WH), MAX_DWH)
    dh = np.minimum(r[:, 3] / np.float32(BW_WH), MAX_DWH)
    pcx = dx * aw + acx
    pcy = dy * ah + acy
    pw = np.exp(dw) * aw
    ph = np.exp(dh) * ah
    x1 = np.clip(pcx - np.float32(0.5) * pw, np.float32(0), np.float32(IMG_W))
    y1 = np.clip(pcy - np.float32(0.5) * ph, np.float32(0), np.float32(IMG_H))
    x2 = np.clip(pcx + np.float32(0.5) * pw, np.float32(0), np.float32(IMG_W))
    y2 = np.clip(pcy + np.float32(0.5) * ph, np.float32(0), np.float32(IMG_H))
    boxes = np.stack([x1, y1, x2, y2], -1)
    sc = np.sqrt(np.maximum(vals, np.float32(0)))
    valid = valid & (x2 - x1 > 0) & (y2 - y1 > 0)

    # class-aware greedy NMS on score-sorted candidates
    K = PRE_NMS_TOP_N
    s = np.where(valid, sc, np.float32(-1.0))
    ob = boxes + (labels.astype(np.float32) * np.float32(10000.0))[:, None]
    area = (ob[:, 2] - ob[:, 0]) * (ob[:, 3] - ob[:, 1])
    ix1 = np.maximum(ob[:, None, 0], ob[None, :, 0])
    iy1 = np.maximum(ob[:, None, 1], ob[None, :, 1])
    ix2 = np.minimum(ob[:, None, 2], ob[None, :, 2])
    iy2 = np.minimum(ob[:, None, 3], ob[None, :, 3])
    inter = np.maximum(ix2 - ix1, np.float32(0)) * np.maximum(
        iy2 - iy1, np.float32(0)
    )
    iou = inter / (area[:, None] + area[None, :] - inter + np.float32(1e-6))
    sup_mat = iou > np.float32(NMS_THRESH)
    keep = valid.copy()
    for i in range(K):
        if keep[i]:
            sup = sup_mat[i].copy()
            sup[: i + 1] = False
            keep &= ~sup
    s2 = np.where(keep, s, np.float32(-1.0))
    # top-100 of s2: s is descending so kept indices in order
    topi = np.lexsort((np.arange(K), -s2))[:POST_TOP_N]
    topv = s2[topi]
    good = topv > 0
    fb = np.where(good[:, None], boxes[topi], np.float32(0))
    fs = np.where(good, topv, np.float32(0)).astype(np.float32)
    fl = np.where(good, labels[topi], np.int32(0)).astype(np.int32)
    return fb.astype(np.float32), fs, fl


def kernel(box_regression, centerness, box_cls, dot_product_logits, anchors,
           pos_map):
    dp_all = np.asarray(dot_product_logits, dtype=np.float32)
    ctr_all = np.asarray(centerness, dtype=np.float32).reshape(N, HW)
    pm = np.asarray(pos_map, dtype=np.float32)
    br = np.asarray(box_regression, dtype=np.float32)
    anc = np.asarray(anchors, dtype=np.float32)

    fused, _ = run_device(dp_all, ctr_all, pm)

    ctr_sig = 1.0 / (1.0 + np.exp(-ctr_all, dtype=np.float32))
    out_b = np.zeros((N, POST_TOP_N, 4), np.float32)
    out_s = np.zeros((N, POST_TOP_N), np.float32)
    out_l = np.zeros((N, POST_TOP_N), np.int32)
    for n in range(N):
        reg = br[n].transpose(1, 2, 0).reshape(HW, 4)
        fb, fs, fl = _host_tail(fused[n], ctr_sig[n], reg, anc)
        out_b[n], out_s[n], out_l[n] = fb, fs, fl
    return out_b, out_s, out_l
